# revision 41
# baseline (speedup 1.0000x reference)
"""Trainium2 Bass kernel for DialecticAttention (16-head RoPE attention +
confidence head), sharded over 8 NeuronCores.

Sharding: data-parallel over batch (2) x head-parallel (4 heads/core).
Core c handles batch b = c // 4 and heads [4*(c%4), 4*(c%4)+4).
Each core computes a partial output projection (its heads' columns of the
Megatron-style row-parallel Wo matmul); the host sums the 4 partials per
batch. The confidence head is folded into the same partial sums via the
precomputed row vector Wc @ Wo.

On-device layout (per core):
  - x is transposed on-chip (PE transpose) to xT [d, l] tiles.
  - q, k are produced directly in transposed layout qT/kT [head_dim, L];
    RoPE is applied there (partition-swap via SBUF-SBUF DMA + DVE muls
    with sign-folded sin).
  - scores are computed transposed: S^T[j, i] = k~ . q~, softmax over the
    partition (j) axis via exp on ScalarE, a ones-matmul column sum, and a
    PE broadcast + DVE reciprocal; normalization is applied to the
    attention output (attn_outT = v.T @ exp(S^T) * recip).
  - all matmuls run as float32r (full-rate fp32 path on TRN2 PE).
"""

import numpy as np

import concourse.bacc as bacc
import concourse.mybir as mybir
import concourse.tile as tile
from concourse.bass_utils import run_bass_kernel_spmd
from concourse.masks import make_identity

F32 = mybir.dt.float32
F32R = mybir.dt.float32r

B, L, D = 2, 2048, 2048
H, HD = 16, 128
P = 128
NCORES = 8
HPC = H // (NCORES // B)      # heads per core = 4
DC = D // P                   # 16 d-chunks
LT = L // P                   # 16 l-tiles
LG = 4                        # l-groups
LGS = L // LG                 # 512
IG = 4                        # i-groups in attention
IGS = L // IG                 # 512
ROWS = HPC * HD               # 512 rows of W per core
SCALE = float(HD) ** -0.5


def _r(ap):
    return ap.bitcast(F32R)


def build_nc():
    nc = bacc.Bacc("TRN2", target_bir_lowering=False, debug=False,
                   num_devices=NCORES)

    xb = nc.dram_tensor("xb", [L, D], F32, kind="ExternalInput").ap()
    wqt = nc.dram_tensor("wqt", [D, ROWS], F32, kind="ExternalInput").ap()
    wkt = nc.dram_tensor("wkt", [D, ROWS], F32, kind="ExternalInput").ap()
    wvt = nc.dram_tensor("wvt", [D, ROWS], F32, kind="ExternalInput").ap()
    wot = nc.dram_tensor("wot", [ROWS, D], F32, kind="ExternalInput").ap()
    wcot = nc.dram_tensor("wcot", [ROWS, 2], F32, kind="ExternalInput").ap()
    cost_d = nc.dram_tensor("cost", [P, L], F32, kind="ExternalInput").ap()
    sints_d = nc.dram_tensor("sints", [P, L], F32, kind="ExternalInput").ap()
    prot_d = nc.dram_tensor("prot", [P, P], F32, kind="ExternalInput").ap()
    out_p = nc.dram_tensor("out_p", [L, D], F32, kind="ExternalOutput").ap()
    conf_p = nc.dram_tensor("conf_p", [L, 1], F32, kind="ExternalOutput").ap()

    xb_r = xb.rearrange("(lt p) d -> lt p d", p=P)
    wq_r = wqt.rearrange("(dc p) n -> dc p n", p=P)
    wk_r = wkt.rearrange("(dc p) n -> dc p n", p=P)
    wv_r = wvt.rearrange("(dc p) n -> dc p n", p=P)
    wot_r = wot.rearrange("(hc p) e -> hc p e", p=P)
    wcot_r = wcot.rearrange("(hc p) o -> p hc o", p=P)  # [P, HPC, 2]
    out_r = out_p.rearrange("(lt p) e -> lt p e", p=P)
    conf_r = conf_p.rearrange("(lt p) o -> p (lt o)", p=P)

    with tile.TileContext(nc) as tc:
        with tc.tile_pool(name="const", bufs=1) as constp:
            cost = constp.tile([P, L], F32)
            sints = constp.tile([P, L], F32)
            ident = constp.tile([P, P], F32)
            ident_r = constp.tile([P, P], F32)
            ones_col = constp.tile([P, 1], F32)
            ones_row = constp.tile([1, P], F32)
            wcot_s = constp.tile([P, HPC, 2], F32)

            prot_s = constp.tile([P, P], F32)
            make_identity(nc, ident)
            nc.vector.tensor_copy(_r(ident_r[:]), ident[:])
            ones_stage = constp.tile([P, 1], F32)
            nc.gpsimd.memset(ones_stage[:], 1.0)
            nc.vector.tensor_copy(_r(ones_col[:]), ones_stage[:])
            nc.gpsimd.memset(ones_row[:], 1.0)

            with tc.tile_pool(name="qkv", bufs=1) as qkvp:
                qt = [qkvp.tile([P, L], F32, tag=f"qt{h}", name=f"qt{h}")
                      for h in range(HPC)]
                kt = [qkvp.tile([P, L], F32, tag=f"kt{h}", name=f"kt{h}")
                      for h in range(HPC)]
                vt = qkvp.tile([P, LT, ROWS], F32, tag="vt")

                # ---------------- Phase 1: transpose + projections -------
                with tc.tile_pool(name="p1", bufs=1) as p1, \
                     tc.tile_pool(name="xsp", bufs=1) as xsp, \
                     tc.tile_pool(name="wsp", bufs=10) as wsp, \
                     tc.tile_pool(name="rtp", bufs=2) as rtp, \
                     tc.tile_pool(name="ptp", bufs=2, space="PSUM") as ptp, \
                     tc.tile_pool(name="ppp", bufs=6, space="PSUM") as ppp:
                    for lg in range(LG):
                        xts = p1.tile([P, DC, LGS], F32, tag="xts")
                        # transpose x rows into xT slice
                        xs = xsp.tile([P, 4, D], F32, tag="xs")
                        for dcq in range(4):
                            dsl = slice(dcq * 512, (dcq + 1) * 512)
                            for t4 in range(4):
                                nc.sync.dma_start(
                                    out=_r(xs[:, t4, dsl]),
                                    in_=_r(xb_r[lg * 4 + t4][:, dsl]))
                        for dc in range(DC):
                            pt = ptp.tile([P, LGS], F32, tag="pt")
                            for t4 in range(4):
                                nc.tensor.matmul(
                                    _r(pt[:, t4 * P:(t4 + 1) * P]),
                                    _r(xs[:, t4, dc * P:(dc + 1) * P]),
                                    _r(ident_r[:]), is_transpose=True,
                                    start=(t4 == 0), stop=(t4 == 3))
                            if dc % 2 == 0:
                                nc.scalar.copy(_r(xts[:, dc, :]), pt[:])
                            else:
                                nc.vector.tensor_copy(_r(xts[:, dc, :]),
                                                      pt[:])
                        if lg == 0:
                            nc.scalar.dma_start(out=cost[:], in_=cost_d)
                            nc.scalar.dma_start(out=sints[:], in_=sints_d)
                            nc.scalar.dma_start(out=_r(prot_s[:]),
                                                in_=_r(prot_d))
                            nc.sync.dma_start(out=_r(wcot_s[:]),
                                              in_=_r(wcot_r))
                        # q/k projections (transposed out) + RoPE
                        for w_r, dst in ((wq_r, qt), (wk_r, kt)):
                            pq = [ppp.tile([P, LGS], F32, tag="pp",
                                            name=f"pq{h}")
                                  for h in range(HPC)]
                            for dc in range(DC):
                                wtile = wsp.tile([P, ROWS], F32, tag="w")
                                if dc % 2 == 0:
                                    nc.sync.dma_start(out=_r(wtile[:]),
                                                      in_=_r(w_r[dc]))
                                else:
                                    nc.scalar.dma_start(out=_r(wtile[:]),
                                                        in_=_r(w_r[dc]))
                                for h in range(HPC):
                                    nc.tensor.matmul(
                                        pq[h][:],
                                        _r(wtile[:, h * HD:(h + 1) * HD]),
                                        _r(xts[:, dc, :]),
                                        start=(dc == 0), stop=(dc == DC - 1))
                            lslice = slice(lg * LGS, (lg + 1) * LGS)
                            for h in range(HPC):
                                praw = rtp.tile([P, LGS], F32, tag="praw")
                                tmp2 = rtp.tile([P, LGS], F32, tag="tmp2")
                                if h % 2 == 0:
                                    nc.scalar.copy(_r(praw[:]), pq[h][:])
                                else:
                                    nc.vector.tensor_copy(_r(praw[:]),
                                                          pq[h][:])
                                pswp = ptp.tile([P, LGS], F32, tag="pt",
                                                name="pswp")
                                nc.tensor.matmul(
                                    pswp[:], _r(prot_s[:]), _r(praw[:]),
                                    start=True, stop=True)
                                nc.vector.tensor_mul(
                                    _r(dst[h][:, lslice]), praw[:],
                                    cost[:, lslice])
                                nc.vector.tensor_mul(
                                    tmp2[:], pswp[:], sints[:, lslice])
                                nc.vector.tensor_add(
                                    _r(dst[h][:, lslice]),
                                    dst[h][:, lslice], tmp2[:])
                        # v projection (natural layout)
                        pv = [ppp.tile([P, ROWS], F32, tag="pp",
                                        name=f"pv{t4}")
                              for t4 in range(4)]
                        for dc in range(DC):
                            wtile = wsp.tile([P, ROWS], F32, tag="w")
                            if dc % 2 == 0:
                                nc.sync.dma_start(out=_r(wtile[:]),
                                                  in_=_r(wv_r[dc]))
                            else:
                                nc.scalar.dma_start(out=_r(wtile[:]),
                                                    in_=_r(wv_r[dc]))
                            for t4 in range(4):
                                nc.tensor.matmul(
                                    pv[t4][:],
                                    _r(xts[:, dc, t4 * P:(t4 + 1) * P]),
                                    _r(wtile[:]),
                                    start=(dc == 0), stop=(dc == DC - 1))
                        for t4 in range(4):
                            if t4 % 2 == 0:
                                nc.scalar.copy(_r(vt[:, lg * 4 + t4, :]),
                                               pv[t4][:])
                            else:
                                nc.vector.tensor_copy(
                                    _r(vt[:, lg * 4 + t4, :]), pv[t4][:])

                # ---------------- Phase 2: attention ---------------------
                with tc.tile_pool(name="aop", bufs=1) as aop, \
                     tc.tile_pool(name="wop", bufs=1) as wop, \
                     tc.tile_pool(name="obp", bufs=3) as obp, \
                     tc.tile_pool(name="cfp", bufs=1) as cfp:
                    ao = [aop.tile([P, L], F32, tag=f"ao{h}", name=f"ao{h}")
                          for h in range(HPC)]
                    conf_sb = cfp.tile([P, LT], F32)
                    wo_s = wop.tile([P, HPC, D], F32)
                    for h in range(HPC):
                        nc.sync.dma_start(out=_r(wo_s[:, h, :]),
                                          in_=_r(wot_r[h]))
                    with tc.tile_pool(name="esp", bufs=5) as esp, \
                         tc.tile_pool(name="smp", bufs=2) as smp, \
                         tc.tile_pool(name="scrp", bufs=1) as scrp, \
                         tc.tile_pool(name="psp", bufs=3, space="PSUM") as psp, \
                         tc.tile_pool(name="pop", bufs=2, space="PSUM") as pop, \
                         tc.tile_pool(name="pcp", bufs=1, space="PSUM") as pcp, \
                         tc.tile_pool(name="po3", bufs=2, space="PSUM") as po3:
                        for ig in range(IG):
                            islice = slice(ig * IGS, (ig + 1) * IGS)
                            for h in range(HPC):
                                pouT = pop.tile([P, IGS], F32, tag="pouT")
                                pcol = pcp.tile([1, IGS], F32, tag="pcol")
                                for cj in range(LT):
                                    ps = psp.tile([P, IGS], F32, tag="ps")
                                    nc.tensor.matmul(
                                        ps[:],
                                        _r(kt[h][:, cj * P:(cj + 1) * P]),
                                        _r(qt[h][:, islice]),
                                        start=True, stop=True)
                                    es = esp.tile([P, IGS], F32, tag="es")
                                    nc.scalar.activation(
                                        _r(es[:]), ps[:],
                                        mybir.ActivationFunctionType.Exp,
                                        scale=SCALE)
                                    nc.tensor.matmul(
                                        pouT[:],
                                        _r(vt[:, cj, h * HD:(h + 1) * HD]),
                                        _r(es[:]),
                                        start=(cj == 0), stop=(cj == LT - 1))
                                    nc.tensor.matmul(
                                        pcol[:], _r(ones_col[:]), _r(es[:]),
                                        start=(cj == 0), stop=(cj == LT - 1))
                                cs = smp.tile([1, IGS], F32, tag="cs")
                                nc.vector.tensor_copy(cs[:], pcol[:])
                                bc = scrp.tile([P, IGS], F32, tag="bc")
                                nc.gpsimd.partition_broadcast(bc[:], cs[:])
                                rbc = smp.tile([P, IGS], F32, tag="rbc")
                                rscr = scrp.tile([P, IGS], F32, tag="rscr")
                                nc.vector.reciprocal_approx_accurate(
                                    rbc[:], bc[:], rscr[:])
                                nc.vector.tensor_mul(
                                    _r(ao[h][:, islice]), pouT[:], rbc[:])
                            # fused output projection for this i-group
                            for eg in range(4):
                                eslice = slice(eg * 512, (eg + 1) * 512)
                                for lt4 in range(4):
                                    lt = ig * 4 + lt4
                                    po = po3.tile([P, 512], F32, tag="po")
                                    for h in range(HPC):
                                        nc.tensor.matmul(
                                            po[:],
                                            _r(ao[h][:, lt * P:(lt + 1) * P]),
                                            _r(wo_s[:, h, eslice]),
                                            start=(h == 0),
                                            stop=(h == HPC - 1))
                                    ob = obp.tile([P, 512], F32, tag="ob")
                                    nc.scalar.copy(ob[:], po[:])
                                    nc.sync.dma_start(
                                        out=out_r[lt][:, eslice], in_=ob[:])
                                    if eg == 0:
                                        pc = po3.tile([P, 2], F32, tag="po",
                                                      name="pc")
                                        for h in range(HPC):
                                            nc.tensor.matmul(
                                                pc[:],
                                                _r(ao[h][:,
                                                         lt * P:(lt + 1) * P]),
                                                _r(wcot_s[:, h, :]),
                                                start=(h == 0),
                                                stop=(h == HPC - 1))
                                        nc.vector.tensor_copy(
                                            conf_sb[:, lt:lt + 1], pc[:, 0:1])

                    nc.sync.dma_start(out=conf_r, in_=conf_sb[:])

    nc.compile()
    return nc


_NC = None


def _get_nc():
    global _NC
    if _NC is None:
        _NC = build_nc()
    return _NC


def _rope_tables():
    inv_freq = 1.0 / (10000.0 ** (np.arange(0, HD, 2, dtype=np.float32) / HD))
    t = np.arange(L, dtype=np.float32)
    freqs = np.outer(t, inv_freq)
    emb = np.concatenate([freqs, freqs], 1)
    cosT = np.ascontiguousarray(np.cos(emb).T.astype(np.float32))
    sinT = np.sin(emb).T.astype(np.float32)
    sinT[:64] *= -1.0
    return cosT, np.ascontiguousarray(sinT)


def make_in_maps(x, Wq, Wk, Wv, Wo, Wc):
    cosT, sinTs = _rope_tables()
    PROT = np.zeros((P, P), np.float32)
    for i in range(P):
        PROT[i ^ 64, i] = 1.0
    Wco = Wc.astype(np.float32) @ Wo.astype(np.float32)   # [1, D]
    in_maps = []
    for c in range(NCORES):
        b = c // (NCORES // B)
        hs = (c % (NCORES // B)) * HPC
        rows = slice(hs * HD, (hs + HPC) * HD)
        in_maps.append({
            "xb": np.ascontiguousarray(x[b]),
            "wqt": np.ascontiguousarray(Wq[rows].T),
            "wkt": np.ascontiguousarray(Wk[rows].T),
            "wvt": np.ascontiguousarray(Wv[rows].T),
            "wot": np.ascontiguousarray(Wo[:, rows].T),
            "wcot": np.ascontiguousarray(
                np.concatenate([Wco[:, rows].T,
                                np.zeros((ROWS, 1), np.float32)], axis=1)),
            "cost": cosT,
            "prot": PROT,
            "sints": sinTs,
        })
    return in_maps


def gather(results):
    out = np.zeros((B, L, D), np.float32)
    logits = np.zeros((B, L, 1), np.float32)
    for c in range(NCORES):
        b = c // (NCORES // B)
        out[b] += results[c]["out_p"]
        logits[b] += results[c]["conf_p"]
    conf = (1.0 / (1.0 + np.exp(-logits.astype(np.float64)))).astype(np.float32)
    return out, conf, logits


def kernel(x, Wq, Wk, Wv, Wo, Wc):
    nc = _get_nc()
    in_maps = make_in_maps(x, Wq, Wk, Wv, Wo, Wc)
    res = run_bass_kernel_spmd(nc, in_maps, list(range(NCORES)))
    return gather(res.results)


# revision 43
# speedup vs baseline: 1.0239x; 1.0239x over previous
"""Trainium2 Bass kernel for DialecticAttention (16-head RoPE attention +
confidence head), sharded over 8 NeuronCores.

Sharding: data-parallel over batch (2) x head-parallel (4 heads/core).
Core c handles batch b = c // 4 and heads [4*(c%4), 4*(c%4)+4).
Each core computes a partial output projection (its heads' columns of the
Megatron-style row-parallel Wo matmul); the host sums the 4 partials per
batch. The confidence head is folded into the same partial sums via the
precomputed row vector Wc @ Wo.

On-device layout (per core):
  - x is transposed on-chip (PE transpose) to xT [d, l] tiles.
  - q, k are produced directly in transposed layout qT/kT [head_dim, L];
    RoPE is applied there (partition-swap via SBUF-SBUF DMA + DVE muls
    with sign-folded sin).
  - scores are computed transposed: S^T[j, i] = k~ . q~, softmax over the
    partition (j) axis via exp on ScalarE, a ones-matmul column sum, and a
    PE broadcast + DVE reciprocal; normalization is applied to the
    attention output (attn_outT = v.T @ exp(S^T) * recip).
  - all matmuls run as float32r (full-rate fp32 path on TRN2 PE).
"""

import numpy as np

import concourse.bacc as bacc
import concourse.mybir as mybir
import concourse.tile as tile
from concourse.bass_utils import run_bass_kernel_spmd
from concourse.masks import make_identity

F32 = mybir.dt.float32
F32R = mybir.dt.float32r

B, L, D = 2, 2048, 2048
H, HD = 16, 128
P = 128
NCORES = 8
HPC = H // (NCORES // B)      # heads per core = 4
DC = D // P                   # 16 d-chunks
LT = L // P                   # 16 l-tiles
LG = 4                        # l-groups
LGS = L // LG                 # 512
IG = 4                        # i-groups in attention
IGS = L // IG                 # 512
ROWS = HPC * HD               # 512 rows of W per core
SCALE = float(HD) ** -0.5


def _r(ap):
    return ap.bitcast(F32R)


def build_nc():
    nc = bacc.Bacc("TRN2", target_bir_lowering=False, debug=False,
                   num_devices=NCORES)

    xb = nc.dram_tensor("xb", [L, D], F32, kind="ExternalInput").ap()
    wqt = nc.dram_tensor("wqt", [D, ROWS], F32, kind="ExternalInput").ap()
    wkt = nc.dram_tensor("wkt", [D, ROWS], F32, kind="ExternalInput").ap()
    wvt = nc.dram_tensor("wvt", [D, ROWS], F32, kind="ExternalInput").ap()
    wot = nc.dram_tensor("wot", [ROWS, D], F32, kind="ExternalInput").ap()
    wcot = nc.dram_tensor("wcot", [ROWS, 2], F32, kind="ExternalInput").ap()
    cost_d = nc.dram_tensor("cost", [P, L], F32, kind="ExternalInput").ap()
    sints_d = nc.dram_tensor("sints", [P, L], F32, kind="ExternalInput").ap()
    prot_d = nc.dram_tensor("prot", [P, P], F32, kind="ExternalInput").ap()
    out_p = nc.dram_tensor("out_p", [L, D], F32, kind="ExternalOutput").ap()
    conf_p = nc.dram_tensor("conf_p", [L, 1], F32, kind="ExternalOutput").ap()

    xb_r = xb.rearrange("(lt p) d -> lt p d", p=P)
    wq_r = wqt.rearrange("(dc p) n -> dc p n", p=P)
    wk_r = wkt.rearrange("(dc p) n -> dc p n", p=P)
    wv_r = wvt.rearrange("(dc p) n -> dc p n", p=P)
    wot_r = wot.rearrange("(hc p) e -> hc p e", p=P)
    wcot_r = wcot.rearrange("(hc p) o -> p hc o", p=P)  # [P, HPC, 2]
    out_r = out_p.rearrange("(lt p) e -> lt p e", p=P)
    conf_r = conf_p.rearrange("(lt p) o -> p (lt o)", p=P)

    with tile.TileContext(nc) as tc:
        with tc.tile_pool(name="const", bufs=1) as constp:
            cost = constp.tile([P, L], F32)
            sints = constp.tile([P, L], F32)
            ident = constp.tile([P, P], F32)
            ident_r = constp.tile([P, P], F32)
            ones_col = constp.tile([P, 1], F32)
            ones_row = constp.tile([1, P], F32)
            wcot_s = constp.tile([P, HPC, 2], F32)

            prot_s = constp.tile([P, P], F32)
            make_identity(nc, ident)
            nc.vector.tensor_copy(_r(ident_r[:]), ident[:])
            ones_stage = constp.tile([P, 1], F32)
            nc.gpsimd.memset(ones_stage[:], 1.0)
            nc.vector.tensor_copy(_r(ones_col[:]), ones_stage[:])
            nc.gpsimd.memset(ones_row[:], 1.0)

            with tc.tile_pool(name="qkv", bufs=1) as qkvp:
                qt = [qkvp.tile([P, L], F32, tag=f"qt{h}", name=f"qt{h}")
                      for h in range(HPC)]
                kt = [qkvp.tile([P, L], F32, tag=f"kt{h}", name=f"kt{h}")
                      for h in range(HPC)]
                vt = qkvp.tile([P, LT, ROWS], F32, tag="vt")

                # ---------------- Phase 1: transpose + projections -------
                with tc.tile_pool(name="p1", bufs=1) as p1, \
                     tc.tile_pool(name="xsp", bufs=1) as xsp, \
                     tc.tile_pool(name="wsp", bufs=10) as wsp, \
                     tc.tile_pool(name="rtp", bufs=2) as rtp, \
                     tc.tile_pool(name="ptp", bufs=3, space="PSUM") as ptp, \
                     tc.tile_pool(name="ppp", bufs=5, space="PSUM") as ppp:
                    for lg in range(LG):
                        xts = p1.tile([P, DC, LGS], F32, tag="xts")
                        # transpose x rows into xT slice
                        xs = xsp.tile([P, 4, D], F32, tag="xs")
                        for dcq in range(4):
                            dsl = slice(dcq * 512, (dcq + 1) * 512)
                            for t4 in range(4):
                                nc.sync.dma_start(
                                    out=_r(xs[:, t4, dsl]),
                                    in_=_r(xb_r[lg * 4 + t4][:, dsl]))
                        for dc in range(DC):
                            pt = ptp.tile([P, LGS], F32, tag="pt")
                            for t4 in range(4):
                                nc.tensor.matmul(
                                    _r(pt[:, t4 * P:(t4 + 1) * P]),
                                    _r(xs[:, t4, dc * P:(dc + 1) * P]),
                                    _r(ident_r[:]), is_transpose=True,
                                    start=(t4 == 0), stop=(t4 == 3))
                            if dc % 2 == 0:
                                nc.scalar.copy(_r(xts[:, dc, :]), pt[:])
                            else:
                                nc.vector.tensor_copy(_r(xts[:, dc, :]),
                                                      pt[:])
                        if lg == 0:
                            nc.scalar.dma_start(out=cost[:], in_=cost_d)
                            nc.scalar.dma_start(out=sints[:], in_=sints_d)
                            nc.scalar.dma_start(out=_r(prot_s[:]),
                                                in_=_r(prot_d))
                            nc.sync.dma_start(out=_r(wcot_s[:]),
                                              in_=_r(wcot_r))
                        # q/k projections (transposed out) + RoPE
                        for w_r, dst in ((wq_r, qt), (wk_r, kt)):
                            pq = [ppp.tile([P, LGS], F32, tag="pp",
                                            name=f"pq{h}")
                                  for h in range(HPC)]
                            for dc in range(DC):
                                wtile = wsp.tile([P, ROWS], F32, tag="w")
                                if dc % 2 == 0:
                                    nc.sync.dma_start(out=_r(wtile[:]),
                                                      in_=_r(w_r[dc]))
                                else:
                                    nc.scalar.dma_start(out=_r(wtile[:]),
                                                        in_=_r(w_r[dc]))
                                for h in range(HPC):
                                    nc.tensor.matmul(
                                        pq[h][:],
                                        _r(wtile[:, h * HD:(h + 1) * HD]),
                                        _r(xts[:, dc, :]),
                                        start=(dc == 0), stop=(dc == DC - 1))
                            lslice = slice(lg * LGS, (lg + 1) * LGS)
                            for h in range(HPC):
                                praw = rtp.tile([P, LGS], F32, tag="praw")
                                tmp2 = rtp.tile([P, LGS], F32, tag="tmp2")
                                if h % 2 == 0:
                                    nc.scalar.copy(_r(praw[:]), pq[h][:])
                                else:
                                    nc.vector.tensor_copy(_r(praw[:]),
                                                          pq[h][:])
                                pswp = ptp.tile([P, LGS], F32, tag="pt",
                                                name="pswp")
                                nc.tensor.matmul(
                                    pswp[:], _r(prot_s[:]), _r(praw[:]),
                                    start=True, stop=True)
                                nc.vector.tensor_mul(
                                    _r(dst[h][:, lslice]), praw[:],
                                    cost[:, lslice])
                                nc.vector.tensor_mul(
                                    tmp2[:], pswp[:], sints[:, lslice])
                                nc.vector.tensor_add(
                                    _r(dst[h][:, lslice]),
                                    dst[h][:, lslice], tmp2[:])
                        # v projection (natural layout)
                        pv = [ppp.tile([P, ROWS], F32, tag="pp",
                                        name=f"pv{t4}")
                              for t4 in range(4)]
                        for dc in range(DC):
                            wtile = wsp.tile([P, ROWS], F32, tag="w")
                            if dc % 2 == 0:
                                nc.sync.dma_start(out=_r(wtile[:]),
                                                  in_=_r(wv_r[dc]))
                            else:
                                nc.scalar.dma_start(out=_r(wtile[:]),
                                                    in_=_r(wv_r[dc]))
                            for t4 in range(4):
                                nc.tensor.matmul(
                                    pv[t4][:],
                                    _r(xts[:, dc, t4 * P:(t4 + 1) * P]),
                                    _r(wtile[:]),
                                    start=(dc == 0), stop=(dc == DC - 1))
                        for t4 in range(4):
                            if t4 % 2 == 0:
                                nc.scalar.copy(_r(vt[:, lg * 4 + t4, :]),
                                               pv[t4][:])
                            else:
                                nc.vector.tensor_copy(
                                    _r(vt[:, lg * 4 + t4, :]), pv[t4][:])

                # ---------------- Phase 2: attention ---------------------
                with tc.tile_pool(name="aop", bufs=1) as aop, \
                     tc.tile_pool(name="wop", bufs=1) as wop, \
                     tc.tile_pool(name="obp", bufs=3) as obp, \
                     tc.tile_pool(name="cfp", bufs=1) as cfp:
                    ao = [aop.tile([P, L], F32, tag=f"ao{h}", name=f"ao{h}")
                          for h in range(HPC)]
                    conf_sb = cfp.tile([P, LT], F32)
                    wo_s = wop.tile([P, HPC, D], F32)
                    for h in range(HPC):
                        nc.sync.dma_start(out=_r(wo_s[:, h, :]),
                                          in_=_r(wot_r[h]))
                    with tc.tile_pool(name="esp", bufs=5) as esp, \
                         tc.tile_pool(name="smp", bufs=2) as smp, \
                         tc.tile_pool(name="scrp", bufs=1) as scrp, \
                         tc.tile_pool(name="psp", bufs=3, space="PSUM") as psp, \
                         tc.tile_pool(name="pop", bufs=2, space="PSUM") as pop, \
                         tc.tile_pool(name="pcp", bufs=1, space="PSUM") as pcp, \
                         tc.tile_pool(name="po3", bufs=2, space="PSUM") as po3:
                        for ig in range(IG):
                            islice = slice(ig * IGS, (ig + 1) * IGS)
                            for h in range(HPC):
                                pouT = pop.tile([P, IGS], F32, tag="pouT")
                                pcol = pcp.tile([1, IGS], F32, tag="pcol")
                                for cj in range(LT):
                                    ps = psp.tile([P, IGS], F32, tag="ps")
                                    nc.tensor.matmul(
                                        ps[:],
                                        _r(kt[h][:, cj * P:(cj + 1) * P]),
                                        _r(qt[h][:, islice]),
                                        start=True, stop=True)
                                    es = esp.tile([P, IGS], F32, tag="es")
                                    nc.scalar.activation(
                                        _r(es[:]), ps[:],
                                        mybir.ActivationFunctionType.Exp,
                                        scale=SCALE)
                                    nc.tensor.matmul(
                                        pouT[:],
                                        _r(vt[:, cj, h * HD:(h + 1) * HD]),
                                        _r(es[:]),
                                        start=(cj == 0), stop=(cj == LT - 1))
                                    nc.tensor.matmul(
                                        pcol[:], _r(ones_col[:]), _r(es[:]),
                                        start=(cj == 0), stop=(cj == LT - 1))
                                cs = smp.tile([1, IGS], F32, tag="cs")
                                nc.vector.tensor_copy(cs[:], pcol[:])
                                bc = scrp.tile([P, IGS], F32, tag="bc")
                                nc.gpsimd.partition_broadcast(bc[:], cs[:])
                                rbc = smp.tile([P, IGS], F32, tag="rbc")
                                rscr = scrp.tile([P, IGS], F32, tag="rscr")
                                nc.vector.reciprocal_approx_accurate(
                                    rbc[:], bc[:], rscr[:])
                                nc.vector.tensor_mul(
                                    _r(ao[h][:, islice]), pouT[:], rbc[:])
                            # fused output projection for this i-group
                            for eg in range(4):
                                eslice = slice(eg * 512, (eg + 1) * 512)
                                for lt4 in range(4):
                                    lt = ig * 4 + lt4
                                    po = po3.tile([P, 512], F32, tag="po")
                                    for h in range(HPC):
                                        nc.tensor.matmul(
                                            po[:],
                                            _r(ao[h][:, lt * P:(lt + 1) * P]),
                                            _r(wo_s[:, h, eslice]),
                                            start=(h == 0),
                                            stop=(h == HPC - 1))
                                    ob = obp.tile([P, 512], F32, tag="ob")
                                    nc.scalar.copy(ob[:], po[:])
                                    nc.sync.dma_start(
                                        out=out_r[lt][:, eslice], in_=ob[:])
                                    if eg == 0:
                                        pc = po3.tile([P, 2], F32, tag="po",
                                                      name="pc")
                                        for h in range(HPC):
                                            nc.tensor.matmul(
                                                pc[:],
                                                _r(ao[h][:,
                                                         lt * P:(lt + 1) * P]),
                                                _r(wcot_s[:, h, :]),
                                                start=(h == 0),
                                                stop=(h == HPC - 1))
                                        nc.vector.tensor_copy(
                                            conf_sb[:, lt:lt + 1], pc[:, 0:1])

                    nc.sync.dma_start(out=conf_r, in_=conf_sb[:])

    nc.compile()
    return nc


_NC = None


def _get_nc():
    global _NC
    if _NC is None:
        _NC = build_nc()
    return _NC


def _rope_tables():
    inv_freq = 1.0 / (10000.0 ** (np.arange(0, HD, 2, dtype=np.float32) / HD))
    t = np.arange(L, dtype=np.float32)
    freqs = np.outer(t, inv_freq)
    emb = np.concatenate([freqs, freqs], 1)
    cosT = np.ascontiguousarray(np.cos(emb).T.astype(np.float32))
    sinT = np.sin(emb).T.astype(np.float32)
    sinT[:64] *= -1.0
    return cosT, np.ascontiguousarray(sinT)


def make_in_maps(x, Wq, Wk, Wv, Wo, Wc):
    cosT, sinTs = _rope_tables()
    PROT = np.zeros((P, P), np.float32)
    for i in range(P):
        PROT[i ^ 64, i] = 1.0
    Wco = Wc.astype(np.float32) @ Wo.astype(np.float32)   # [1, D]
    in_maps = []
    for c in range(NCORES):
        b = c // (NCORES // B)
        hs = (c % (NCORES // B)) * HPC
        rows = slice(hs * HD, (hs + HPC) * HD)
        in_maps.append({
            "xb": np.ascontiguousarray(x[b]),
            "wqt": np.ascontiguousarray(Wq[rows].T),
            "wkt": np.ascontiguousarray(Wk[rows].T),
            "wvt": np.ascontiguousarray(Wv[rows].T),
            "wot": np.ascontiguousarray(Wo[:, rows].T),
            "wcot": np.ascontiguousarray(
                np.concatenate([Wco[:, rows].T,
                                np.zeros((ROWS, 1), np.float32)], axis=1)),
            "cost": cosT,
            "prot": PROT,
            "sints": sinTs,
        })
    return in_maps


def gather(results):
    out = np.zeros((B, L, D), np.float32)
    logits = np.zeros((B, L, 1), np.float32)
    for c in range(NCORES):
        b = c // (NCORES // B)
        out[b] += results[c]["out_p"]
        logits[b] += results[c]["conf_p"]
    conf = (1.0 / (1.0 + np.exp(-logits.astype(np.float64)))).astype(np.float32)
    return out, conf, logits


def kernel(x, Wq, Wk, Wv, Wo, Wc):
    nc = _get_nc()
    in_maps = make_in_maps(x, Wq, Wk, Wv, Wo, Wc)
    res = run_bass_kernel_spmd(nc, in_maps, list(range(NCORES)))
    return gather(res.results)
